# revision 21
# baseline (speedup 1.0000x reference)
"""Self-attention kernel for TRN2, data-parallel over batch (8 cores).

FP8 (e4m3) DoubleRow version: all matmuls run with perf_mode=DoubleRow
(2 fp8 MACs/PE/cycle, contraction 256 per instruction).

Per core (one batch element):
  x8 (host-quantized fp8) -> x^T via regular fp8 matmuls against identity,
  q^T/k^T projections (u on partitions) and v (row-major) via DoubleRow,
  scores computed transposed in pair layout p[s-pair][ko, t] = exp(.-ln64),
  PV with stationary v-pairs / moving p-pairs -> a^T directly in PSUM
  (unnormalized), row sums via ones-pair stationary matmul -> [1, t] PSUM,
  sums transposed to per-partition layout with 4 tiny matmuls (x16),
  reciprocal [128,4], normalization fused into the output epilogue:
  y = yps * rcpT + x  (scalar_tensor_tensor), residual x in fp32.

Host side: weights prescaled x16 and quantized to fp8 (e4m3 bit-compatible
with TRN fp8e4 for |v|<=240), x quantized to fp8 for the compute path
(f32 copy loaded late for the residual), bv/ba folded into an effective
residual bias.
"""

import numpy as np
import ml_dtypes

import concourse.bass as bass
import concourse.mybir as mybir
import concourse.tile as tile
from concourse import bacc
from concourse.bass import ds, ts
from concourse.bass_utils import run_bass_kernel_spmd
from concourse.masks import make_identity

F32 = mybir.dt.float32
BF16 = mybir.dt.bfloat16
F8 = mybir.dt.float8e4
AF = mybir.ActivationFunctionType
DR = mybir.MatmulPerfMode.DoubleRow
MUL = mybir.AluOpType.mult
ADD = mybir.AluOpType.add

B, T, C, U, P = 8, 2048, 512, 256, 128
TC = T // P      # 16 row tiles
CCH = C // P     # 4 c-chunks
UCH = U // P     # 2 u-chunks
NPAIR = TC // 2  # 8 s-pairs
TBLK = 512       # t-block for attention
NTB = T // TBLK  # 4
SCALE = 1.0 / float(np.sqrt(U))
LOGSHIFT = float(np.log(64.0))
WS = 16.0        # host-side weight prescale

_cache = {}


def _build_kernel(tc, with_bias):
    nc = tc.nc
    x = nc.dram_tensor("x", [T, C], F32, kind="ExternalInput").ap()
    x8d = nc.dram_tensor("x8", [T, C], F8, kind="ExternalInput").ap()
    Wq8d = nc.dram_tensor("Wq8", [P, CCH, U], F8, kind="ExternalInput").ap()
    Wk8d = nc.dram_tensor("Wk8", [P, CCH, U], F8, kind="ExternalInput").ap()
    Wv8d = nc.dram_tensor("Wv8", [P, CCH, U], F8, kind="ExternalInput").ap()
    Wa8d = nc.dram_tensor("Wa8", [P, UCH, C], F8, kind="ExternalInput").ap()
    bq = nc.dram_tensor("bq", [U], F32, kind="ExternalInput").ap()
    bk = nc.dram_tensor("bk", [U], F32, kind="ExternalInput").ap()
    ba = nc.dram_tensor("ba", [C], F32, kind="ExternalInput").ap()
    out = nc.dram_tensor("out", [T, C], F32, kind="ExternalOutput").ap()

    consts = tc.alloc_tile_pool(name="consts", bufs=1)
    persist = consts

    ident = consts.tile([P, P], F8)
    make_identity(nc, ident)
    # block identity for paired DoubleRow transposes: [I|0] / [0|I]
    ident2 = consts.tile([P, 2, 2 * P], F8)
    nc.vector.memset(ident2, 0.0)
    make_identity(nc, ident2[:, 0, 0:P], nomemset=True)
    make_identity(nc, ident2[:, 1, P:2 * P], nomemset=True)
    ones3 = consts.tile([P, 2, 16], F8)
    nc.vector.memset(ones3, 1.0)
    nbias = consts.tile([P, 1], F32)
    nc.vector.memset(nbias, -LOGSHIFT)
    sixteen = consts.tile([1, 1], BF16)
    nc.vector.memset(sixteen, 16.0)

    # fp8 weights: straight DMA, pre-arranged on host
    Wq8 = consts.tile([P, CCH, U], F8)
    nc.gpsimd.dma_start(out=Wq8, in_=Wq8d)
    Wk8 = consts.tile([P, CCH, U], F8)
    nc.gpsimd.dma_start(out=Wk8, in_=Wk8d)
    Wv8 = consts.tile([P, CCH, U], F8)
    nc.gpsimd.dma_start(out=Wv8, in_=Wv8d)
    Wa8 = consts.tile([P, UCH, C], F8)
    nc.gpsimd.dma_start(out=Wa8, in_=Wa8d)
    bq_sb = consts.tile([P, UCH], F32)
    nc.sync.dma_start(out=bq_sb, in_=bq.rearrange("(uc p) -> p uc", p=P))
    bk_sb = consts.tile([P, UCH], F32)
    nc.sync.dma_start(out=bk_sb, in_=bk.rearrange("(uc p) -> p uc", p=P))
    if with_bias:
        ba_row = consts.tile([1, C], F32)
        nc.sync.dma_start(out=ba_row, in_=ba[None, :])
        ba_bc = consts.tile([P, C], F32)
        nc.gpsimd.partition_broadcast(ba_bc, ba_row)

    # persistent layout tensors
    x8_sb = persist.tile([P, TC, C], F8)      # x fp8 (transpose source)
    x_sb = persist.tile([P, TC, C], F32)      # x rows fp32 (residual, late)
    xT8 = persist.tile([P, CCH, T], F8)       # x^T  (c on partitions)
    qT8 = persist.tile([P, UCH, T], F8)       # q^T  (u on partitions)
    kT8 = persist.tile([P, UCH, T], F8)       # k^T
    v8 = persist.tile([P, TC, U], F8)         # v row-major
    aT8 = persist.tile([P, UCH, T], F8)       # a^T (unnormalized)

    # HAM warm-up (keeps PE busy through the initial DMA wait)
    with tc.tile_pool(name="warm", bufs=1, space="PSUM") as warm_pool:
        wtile = warm_pool.tile([P, P], F32, name="warmup")
        for i in range(14):
            nc.tensor.matmul(wtile, lhsT=ident, rhs=ident,
                             start=(i == 0), stop=(i == 13))

    # compute-path x (fp8, 64KB per tile) — the critical input
    for tt in range(TC):
        eng = nc.sync if tt % 2 == 0 else nc.scalar
        eng.dma_start(out=x8_sb[:, tt, :], in_=x8d[ts(tt, P), :])

    # --- phase 1: x^T via paired DoubleRow matmuls against [I|0],[0|I] ---
    with tc.tile_pool(name="tpsum", bufs=2, space="PSUM") as tpsum:
        for th in range(TC // 2):
            tps = tpsum.tile([P, CCH, 2 * P], F32, tag="tps")
            for cc in range(CCH):
                nc.tensor.matmul(tps[:, cc, :],
                                 lhsT=x8_sb[:, 2 * th:2 * th + 2, ts(cc, P)],
                                 rhs=ident2, start=True, stop=True,
                                 perf_mode=DR)
            nc.scalar.activation(out=xT8[:, :, ds(th * 2 * P, 2 * P)],
                                 in_=tps, func=AF.Identity)

    # --- phase 2: projections (DoubleRow over c) ---
    with tc.tile_pool(name="wpsum", bufs=2, space="PSUM") as wpsum:
        vpsum = wpsum
        for (W8, b_sb, dst) in ((Wk8, bk_sb, kT8), (Wq8, bq_sb, qT8)):
            for tb in range(NTB):
                for uc in range(UCH):
                    ps = wpsum.tile([P, TBLK], F32, tag="wps")
                    for j in range(2):
                        nc.tensor.matmul(
                            ps,
                            lhsT=W8[:, 2 * j:2 * j + 2, ts(uc, P)],
                            rhs=xT8[:, 2 * j:2 * j + 2, ds(tb * TBLK, TBLK)],
                            start=(j == 0), stop=(j == 1), perf_mode=DR)
                    nc.vector.tensor_scalar(
                        out=dst[:, uc, ds(tb * TBLK, TBLK)], in0=ps,
                        scalar1=1.0 / WS, scalar2=b_sb[:, uc:uc + 1],
                        op0=MUL, op1=ADD)
        for tt in range(TC):
            vps = vpsum.tile([P, U], F32, tag="vps")
            for j in range(2):
                nc.tensor.matmul(
                    vps,
                    lhsT=xT8[:, 2 * j:2 * j + 2, ts(tt, P)],
                    rhs=Wv8[:, 2 * j:2 * j + 2, :],
                    start=(j == 0), stop=(j == 1), perf_mode=DR)
            nc.scalar.activation(out=v8[:, tt, :], in_=vps,
                                 func=AF.Identity, scale=1.0 / WS)

    # residual x fp32: issued late so it doesn't fight the critical loads;
    # executes during the attention phase, needed first by finish(tb0)
    for tt in range(TC):
        nc.sync.dma_start(out=x_sb[:, tt, :], in_=x[ts(tt, P), :])
        if with_bias:
            nc.vector.tensor_add(out=x_sb[:, tt, :],
                                 in0=x_sb[:, tt, :], in1=ba_bc)

    # --- phase 3: attention ---
    # PSUM banks: score-pairs 2x2 + aT 2 + sums/sT shared 1 + yps 1 = 8
    psA = tc.alloc_tile_pool(name="psA", bufs=2, space="PSUM")
    psB = tc.alloc_tile_pool(name="psB", bufs=1, space="PSUM")
    spsum = apsum = psA
    smpsum = ypsum = psB
    work = tc.alloc_tile_pool(name="work", bufs=8)
    p_pool = smb_pool = rcp_pool = y_pool = work

    def finish_one(tb, rcpT, tsl, pool=None, tag="yps"):
        r = tb * NTB + tsl
        yps = (pool or ypsum).tile([P, C], F32, tag=tag,
                                   name=f"yps{tb}_{tsl}")
        nc.tensor.matmul(yps, lhsT=aT8[:, :, ts(r, P)], rhs=Wa8,
                         start=True, stop=True, perf_mode=DR)
        y_sb = y_pool.tile([P, C], F32, tag="ysb")
        nc.vector.scalar_tensor_tensor(
            out=y_sb, in0=yps, scalar=rcpT[:, tsl:tsl + 1],
            in1=x_sb[:, r, :], op0=MUL, op1=ADD)
        nc.sync.dma_start(out=out[ts(r, P), :], in_=y_sb)

    deferred = None
    for tb in range(NTB):
        ap_tiles = [apsum.tile([P, TBLK], F32, tag="aps",
                               name=f"aps{tb}_{uc}") for uc in range(UCH)]
        # sums row [1,t] and its transpose [P,4] share one bank: the sT
        # matmuls overwrite bytes 0-15 only after sums_bf has been read
        sums_ps = smpsum.tile([P, TBLK], F32, tag="sums", name=f"sums{tb}")
        pps = []

        def pv_pair(j):
            for uc in range(UCH):
                nc.tensor.matmul(
                    ap_tiles[uc],
                    lhsT=v8[:, 2 * j:2 * j + 2, ts(uc, P)],
                    rhs=pps[j],
                    start=(j == 0), stop=(j == NPAIR - 1), perf_mode=DR)
            nc.tensor.matmul(
                sums_ps[0:1, :], lhsT=ones3[:, :, 0:1], rhs=pps[j],
                start=(j == 0), stop=(j == NPAIR - 1), perf_mode=DR)

        for sc in range(TC):
            j = sc // 2
            if sc % 2 == 0:
                pps.append(p_pool.tile([P, 2, TBLK], F8, tag="pp",
                                       name=f"pp{tb}_{j}"))
                sp_pair = spsum.tile([P, 2, TBLK], F32, tag="sps",
                                     name=f"sps{tb}_{j}")
            nc.tensor.matmul(sp_pair[:, sc % 2, :], lhsT=kT8[:, :, ts(sc, P)],
                             rhs=qT8[:, :, ds(tb * TBLK, TBLK)],
                             start=True, stop=True, perf_mode=DR)
            if j == 0:
                nc.scalar.activation(out=pps[j][:, sc % 2, :],
                                     in_=sp_pair[:, sc % 2, :],
                                     func=AF.Exp, scale=SCALE, bias=nbias)
            if sc % 2 == 1:
                if j > 0:
                    nc.scalar.activation(out=pps[j], in_=sp_pair,
                                         func=AF.Exp, scale=SCALE, bias=nbias)
                if j >= 1:
                    pv_pair(j - 1)
                # spread the previous block's output projection through
                # this block so single-buffered yps never stalls the PE
                if deferred is not None and j < NTB:
                    finish_one(*deferred, tsl=j)
        pv_pair(NPAIR - 1)

        # drain a^T unnormalized (normalization folded into the epilogue)
        for uc in range(UCH):
            nc.vector.tensor_copy(out=aT8[:, uc, ds(tb * TBLK, TBLK)],
                                  in_=ap_tiles[uc])
        # row sums -> per-partition reciprocal
        sums_bf = smb_pool.tile([1, TBLK], BF16, tag="smb")
        nc.vector.tensor_copy(out=sums_bf, in_=sums_ps[0:1, :])
        for tsl in range(NTB):
            nc.tensor.matmul(sums_ps[:, tsl:tsl + 1],
                             lhsT=sums_bf[:, ts(tsl, P)], rhs=sixteen,
                             start=True, stop=True)
        rcpT = rcp_pool.tile([P, NTB], F32, tag="rcpT")
        nc.vector.reciprocal(rcpT, sums_ps[:, 0:NTB])

        deferred = (tb, rcpT)
    # final block: alternate PSUM pools (apsum is free now) so the four
    # tail projections double-buffer
    for tsl in range(NTB):
        if tsl % 2 == 0:
            finish_one(*deferred, tsl=tsl)
        else:
            finish_one(*deferred, tsl=tsl, pool=apsum, tag="aps")

    for pool in (work, psB, psA, consts):
        pool.release()


def _get_nc(with_bias=False):
    key = ("nc", with_bias)
    if key not in _cache:
        nc = bacc.Bacc("TRN2", target_bir_lowering=False, debug=False)
        with tile.TileContext(nc) as tc:
            _build_kernel(tc, with_bias)
        nc.compile()
        _cache[key] = nc
    return _cache[key]


def _to_f8(a):
    return np.clip(a, -240.0, 240.0).astype(ml_dtypes.float8_e4m3)


def _prep_inputs(inputs):
    f32 = lambda a: np.ascontiguousarray(np.asarray(a, dtype=np.float32))
    Wq, Wk, Wv, Wa = (f32(inputs[k]) for k in ("Wq", "Wk", "Wv", "Wa"))
    bq, bk, bv, ba = (f32(inputs[k]) for k in ("bq", "bk", "bv", "ba"))
    ba_eff = (bv @ Wa + ba).astype(np.float32)
    with_bias = bool(np.any(bq) or np.any(bk) or np.any(ba_eff))
    WSf = np.float32(WS)
    # [C, U] -> [P, CCH, U]; [U, C] -> [P, UCH, C]
    Wq8 = _to_f8((Wq * WSf).reshape(CCH, P, U).transpose(1, 0, 2))
    Wk8 = _to_f8((Wk * WSf).reshape(CCH, P, U).transpose(1, 0, 2))
    Wv8 = _to_f8((Wv * WSf).reshape(CCH, P, U).transpose(1, 0, 2))
    Wa8 = _to_f8((Wa * WSf).reshape(UCH, P, C).transpose(1, 0, 2))
    shared = {
        "Wq8": np.ascontiguousarray(Wq8),
        "Wk8": np.ascontiguousarray(Wk8),
        "Wv8": np.ascontiguousarray(Wv8),
        "Wa8": np.ascontiguousarray(Wa8),
        "bq": bq, "bk": bk, "ba": np.ascontiguousarray(ba_eff),
    }
    return shared, with_bias


def kernel(**inputs):
    shared, with_bias = _prep_inputs(inputs)
    nc = _get_nc(with_bias)
    xs = np.ascontiguousarray(np.asarray(inputs["x"], dtype=np.float32))
    in_maps = [dict(shared, x=xs[b], x8=np.ascontiguousarray(_to_f8(xs[b])))
               for b in range(B)]
    res = run_bass_kernel_spmd(nc, in_maps, core_ids=list(range(B)))
    return np.stack([res.results[b]["out"] for b in range(B)], axis=0)


# revision 22
# speedup vs baseline: 1.0323x; 1.0323x over previous
"""Self-attention kernel for TRN2, data-parallel over batch (8 cores).

FP8 (e4m3) DoubleRow version: all matmuls run with perf_mode=DoubleRow
(2 fp8 MACs/PE/cycle, contraction 256 per instruction).

Per core (one batch element):
  x8 (host-quantized fp8) -> x^T via regular fp8 matmuls against identity,
  q^T/k^T projections (u on partitions) and v (row-major) via DoubleRow,
  scores computed transposed in pair layout p[s-pair][ko, t] = exp(.-ln64),
  PV with stationary v-pairs / moving p-pairs -> a^T directly in PSUM
  (unnormalized), row sums via ones-pair stationary matmul -> [1, t] PSUM,
  sums transposed to per-partition layout with 4 tiny matmuls (x16),
  reciprocal [128,4], normalization fused into the output epilogue:
  y = yps * rcpT + x  (scalar_tensor_tensor), residual x in fp32.

Host side: weights prescaled x16 and quantized to fp8 (e4m3 bit-compatible
with TRN fp8e4 for |v|<=240), x quantized to fp8 for the compute path
(f32 copy loaded late for the residual), bv/ba folded into an effective
residual bias.
"""

import numpy as np
import ml_dtypes

import concourse.bass as bass
import concourse.mybir as mybir
import concourse.tile as tile
from concourse import bacc
from concourse.bass import ds, ts
from concourse.bass_utils import run_bass_kernel_spmd
from concourse.masks import make_identity

F32 = mybir.dt.float32
BF16 = mybir.dt.bfloat16
F8 = mybir.dt.float8e4
AF = mybir.ActivationFunctionType
DR = mybir.MatmulPerfMode.DoubleRow
MUL = mybir.AluOpType.mult
ADD = mybir.AluOpType.add

B, T, C, U, P = 8, 2048, 512, 256, 128
TC = T // P      # 16 row tiles
CCH = C // P     # 4 c-chunks
UCH = U // P     # 2 u-chunks
NPAIR = TC // 2  # 8 s-pairs
TBLK = 512       # t-block for attention
NTB = T // TBLK  # 4
SCALE = 1.0 / float(np.sqrt(U))
LOGSHIFT = float(np.log(64.0))
WS = 16.0        # host-side weight prescale

_cache = {}


def _build_kernel(tc, with_bias):
    nc = tc.nc
    x = nc.dram_tensor("x", [T, C], F32, kind="ExternalInput").ap()
    x8d = nc.dram_tensor("x8", [T, C], F8, kind="ExternalInput").ap()
    Wq8d = nc.dram_tensor("Wq8", [P, CCH, U], F8, kind="ExternalInput").ap()
    Wk8d = nc.dram_tensor("Wk8", [P, CCH, U], F8, kind="ExternalInput").ap()
    Wv8d = nc.dram_tensor("Wv8", [P, CCH, U], F8, kind="ExternalInput").ap()
    Wa8d = nc.dram_tensor("Wa8", [P, UCH, C], F8, kind="ExternalInput").ap()
    bq = nc.dram_tensor("bq", [U], F32, kind="ExternalInput").ap()
    bk = nc.dram_tensor("bk", [U], F32, kind="ExternalInput").ap()
    ba = nc.dram_tensor("ba", [C], F32, kind="ExternalInput").ap()
    out = nc.dram_tensor("out", [T, C], F32, kind="ExternalOutput").ap()

    consts = tc.alloc_tile_pool(name="consts", bufs=1)
    persist = consts

    ident = consts.tile([P, P], F8)
    make_identity(nc, ident)
    # block identity for paired DoubleRow transposes: [I|0] / [0|I]
    ident2 = consts.tile([P, 2, 2 * P], F8)
    nc.vector.memset(ident2, 0.0)
    make_identity(nc, ident2[:, 0, 0:P], nomemset=True)
    make_identity(nc, ident2[:, 1, P:2 * P], nomemset=True)
    ones3 = consts.tile([P, 2, 16], F8)
    nc.vector.memset(ones3, 1.0)
    nbias = consts.tile([P, 1], F32)
    nc.vector.memset(nbias, -LOGSHIFT)
    sixteen = consts.tile([1, 1], BF16)
    nc.vector.memset(sixteen, 16.0)

    # fp8 weights: straight DMA, pre-arranged on host
    Wq8 = consts.tile([P, CCH, U], F8)
    nc.gpsimd.dma_start(out=Wq8, in_=Wq8d)
    Wk8 = consts.tile([P, CCH, U], F8)
    nc.gpsimd.dma_start(out=Wk8, in_=Wk8d)
    Wv8 = consts.tile([P, CCH, U], F8)
    nc.gpsimd.dma_start(out=Wv8, in_=Wv8d)
    Wa8 = consts.tile([P, UCH, C], F8)
    nc.gpsimd.dma_start(out=Wa8, in_=Wa8d)
    bq_sb = consts.tile([P, UCH], F32)
    nc.sync.dma_start(out=bq_sb, in_=bq.rearrange("(uc p) -> p uc", p=P))
    bk_sb = consts.tile([P, UCH], F32)
    nc.sync.dma_start(out=bk_sb, in_=bk.rearrange("(uc p) -> p uc", p=P))
    if with_bias:
        ba_row = consts.tile([1, C], F32)
        nc.sync.dma_start(out=ba_row, in_=ba[None, :])
        ba_bc = consts.tile([P, C], F32)
        nc.gpsimd.partition_broadcast(ba_bc, ba_row)

    # persistent layout tensors
    x8_sb = persist.tile([P, TC, C], F8)      # x fp8 (transpose source)
    x_sb = persist.tile([P, TC, C], F32)      # x rows fp32 (residual, late)
    xT8 = persist.tile([P, CCH, T], F8)       # x^T  (c on partitions)
    qT8 = persist.tile([P, UCH, T], F8)       # q^T  (u on partitions)
    kT8 = persist.tile([P, UCH, T], F8)       # k^T
    v8 = persist.tile([P, TC, U], F8)         # v row-major
    aT8 = persist.tile([P, UCH, T], F8)       # a^T (unnormalized)

    # HAM warm-up (keeps PE busy through the initial DMA wait)
    with tc.tile_pool(name="warm", bufs=1, space="PSUM") as warm_pool:
        wtile = warm_pool.tile([P, P], F32, name="warmup")
        for i in range(14):
            nc.tensor.matmul(wtile, lhsT=ident, rhs=ident,
                             start=(i == 0), stop=(i == 13))

    # compute-path x (fp8, 64KB per tile) — the critical input
    for tt in range(TC):
        eng = nc.sync if tt % 2 == 0 else nc.scalar
        eng.dma_start(out=x8_sb[:, tt, :], in_=x8d[ts(tt, P), :])

    # --- phase 1: x^T via paired DoubleRow matmuls against [I|0],[0|I] ---
    with tc.tile_pool(name="tpsum", bufs=2, space="PSUM") as tpsum:
        for th in range(TC // 2):
            tps = tpsum.tile([P, CCH, 2 * P], F32, tag="tps")
            for cc in range(CCH):
                nc.tensor.matmul(tps[:, cc, :],
                                 lhsT=x8_sb[:, 2 * th:2 * th + 2, ts(cc, P)],
                                 rhs=ident2, start=True, stop=True,
                                 perf_mode=DR)
            nc.scalar.activation(out=xT8[:, :, ds(th * 2 * P, 2 * P)],
                                 in_=tps, func=AF.Identity)

    # --- phase 2: projections (DoubleRow over c) ---
    with tc.tile_pool(name="wpsum", bufs=2, space="PSUM") as wpsum:
        vpsum = wpsum
        for (W8, b_sb, dst) in ((Wk8, bk_sb, kT8), (Wq8, bq_sb, qT8)):
            for tb in range(NTB):
                for uc in range(UCH):
                    ps = wpsum.tile([P, TBLK], F32, tag="wps")
                    for j in range(2):
                        nc.tensor.matmul(
                            ps,
                            lhsT=W8[:, 2 * j:2 * j + 2, ts(uc, P)],
                            rhs=xT8[:, 2 * j:2 * j + 2, ds(tb * TBLK, TBLK)],
                            start=(j == 0), stop=(j == 1), perf_mode=DR)
                    nc.vector.tensor_scalar(
                        out=dst[:, uc, ds(tb * TBLK, TBLK)], in0=ps,
                        scalar1=1.0 / WS, scalar2=b_sb[:, uc:uc + 1],
                        op0=MUL, op1=ADD)
        for tt in range(TC):
            vps = vpsum.tile([P, U], F32, tag="vps")
            for j in range(2):
                nc.tensor.matmul(
                    vps,
                    lhsT=xT8[:, 2 * j:2 * j + 2, ts(tt, P)],
                    rhs=Wv8[:, 2 * j:2 * j + 2, :],
                    start=(j == 0), stop=(j == 1), perf_mode=DR)
            nc.scalar.activation(out=v8[:, tt, :], in_=vps,
                                 func=AF.Identity, scale=1.0 / WS)

    # residual x fp32: issued late so it doesn't fight the critical loads;
    # executes during the attention phase, needed first by finish(tb0)
    for tt in range(TC):
        nc.sync.dma_start(out=x_sb[:, tt, :], in_=x[ts(tt, P), :])
        if with_bias:
            nc.vector.tensor_add(out=x_sb[:, tt, :],
                                 in0=x_sb[:, tt, :], in1=ba_bc)

    # --- phase 3: attention ---
    # PSUM banks: score-pairs 2x2 + aT 2 + sums/sT shared 1 + yps 1 = 8
    psA = tc.alloc_tile_pool(name="psA", bufs=2, space="PSUM")
    psB = tc.alloc_tile_pool(name="psB", bufs=1, space="PSUM")
    spsum = apsum = psA
    smpsum = ypsum = psB
    work = tc.alloc_tile_pool(name="work", bufs=8)
    p_pool = smb_pool = rcp_pool = y_pool = work

    def finish_one(tb, rcpT, tsl, pool=None, tag="yps"):
        r = tb * NTB + tsl
        yps = (pool or ypsum).tile([P, C], F32, tag=tag,
                                   name=f"yps{tb}_{tsl}")
        nc.tensor.matmul(yps, lhsT=aT8[:, :, ts(r, P)], rhs=Wa8,
                         start=True, stop=True, perf_mode=DR)
        y_sb = y_pool.tile([P, C], F32, tag="ysb")
        nc.vector.scalar_tensor_tensor(
            out=y_sb, in0=yps, scalar=rcpT[:, tsl:tsl + 1],
            in1=x_sb[:, r, :], op0=MUL, op1=ADD)
        nc.sync.dma_start(out=out[ts(r, P), :], in_=y_sb)

    deferred = None
    for tb in range(NTB):
        ap_tiles = [apsum.tile([P, TBLK], F32, tag="aps",
                               name=f"aps{tb}_{uc}") for uc in range(UCH)]
        # sums row [1,t] and its transpose [P,4] share one bank: the sT
        # matmuls overwrite bytes 0-15 only after sums_bf has been read
        sums_ps = smpsum.tile([P, TBLK], F32, tag="sums", name=f"sums{tb}")
        pps = []

        def pv_pair(j):
            for uc in range(UCH):
                nc.tensor.matmul(
                    ap_tiles[uc],
                    lhsT=v8[:, 2 * j:2 * j + 2, ts(uc, P)],
                    rhs=pps[j],
                    start=(j == 0), stop=(j == NPAIR - 1), perf_mode=DR)
            nc.tensor.matmul(
                sums_ps[0:1, :], lhsT=ones3[:, :, 0:1], rhs=pps[j],
                start=(j == 0), stop=(j == NPAIR - 1), perf_mode=DR)

        for sc in range(TC):
            j = sc // 2
            if sc % 2 == 0:
                pps.append(p_pool.tile([P, 2, TBLK], F8, tag="pp",
                                       name=f"pp{tb}_{j}"))
                sp_pair = spsum.tile([P, 2, TBLK], F32, tag="sps",
                                     name=f"sps{tb}_{j}")
            nc.tensor.matmul(sp_pair[:, sc % 2, :], lhsT=kT8[:, :, ts(sc, P)],
                             rhs=qT8[:, :, ds(tb * TBLK, TBLK)],
                             start=True, stop=True, perf_mode=DR)
            if sc % 2 == 1:
                nc.scalar.activation(out=pps[j], in_=sp_pair,
                                     func=AF.Exp, scale=SCALE, bias=nbias)
                if j >= 1:
                    pv_pair(j - 1)
                # spread the previous block's output projection through
                # this block so single-buffered yps never stalls the PE
                if deferred is not None and j < NTB:
                    finish_one(*deferred, tsl=j)
        pv_pair(NPAIR - 1)

        # drain a^T unnormalized (normalization folded into the epilogue)
        for uc in range(UCH):
            nc.vector.tensor_copy(out=aT8[:, uc, ds(tb * TBLK, TBLK)],
                                  in_=ap_tiles[uc])
        # row sums -> per-partition reciprocal
        sums_bf = smb_pool.tile([1, TBLK], BF16, tag="smb")
        nc.vector.tensor_copy(out=sums_bf, in_=sums_ps[0:1, :])
        for tsl in range(NTB):
            nc.tensor.matmul(sums_ps[:, tsl:tsl + 1],
                             lhsT=sums_bf[:, ts(tsl, P)], rhs=sixteen,
                             start=True, stop=True)
        rcpT = rcp_pool.tile([P, NTB], F32, tag="rcpT")
        nc.vector.reciprocal(rcpT, sums_ps[:, 0:NTB])

        deferred = (tb, rcpT)
    # final block: alternate PSUM pools (apsum is free now) so the four
    # tail projections double-buffer
    for tsl in range(NTB):
        if tsl % 2 == 0:
            finish_one(*deferred, tsl=tsl)
        else:
            finish_one(*deferred, tsl=tsl, pool=apsum, tag="aps")

    for pool in (work, psB, psA, consts):
        pool.release()


def _get_nc(with_bias=False):
    key = ("nc", with_bias)
    if key not in _cache:
        nc = bacc.Bacc("TRN2", target_bir_lowering=False, debug=False)
        with tile.TileContext(nc) as tc:
            _build_kernel(tc, with_bias)
        nc.compile()
        _cache[key] = nc
    return _cache[key]


def _to_f8(a):
    return np.clip(a, -240.0, 240.0).astype(ml_dtypes.float8_e4m3)


def _prep_inputs(inputs):
    f32 = lambda a: np.ascontiguousarray(np.asarray(a, dtype=np.float32))
    Wq, Wk, Wv, Wa = (f32(inputs[k]) for k in ("Wq", "Wk", "Wv", "Wa"))
    bq, bk, bv, ba = (f32(inputs[k]) for k in ("bq", "bk", "bv", "ba"))
    ba_eff = (bv @ Wa + ba).astype(np.float32)
    with_bias = bool(np.any(bq) or np.any(bk) or np.any(ba_eff))
    WSf = np.float32(WS)
    # [C, U] -> [P, CCH, U]; [U, C] -> [P, UCH, C]
    Wq8 = _to_f8((Wq * WSf).reshape(CCH, P, U).transpose(1, 0, 2))
    Wk8 = _to_f8((Wk * WSf).reshape(CCH, P, U).transpose(1, 0, 2))
    Wv8 = _to_f8((Wv * WSf).reshape(CCH, P, U).transpose(1, 0, 2))
    Wa8 = _to_f8((Wa * WSf).reshape(UCH, P, C).transpose(1, 0, 2))
    shared = {
        "Wq8": np.ascontiguousarray(Wq8),
        "Wk8": np.ascontiguousarray(Wk8),
        "Wv8": np.ascontiguousarray(Wv8),
        "Wa8": np.ascontiguousarray(Wa8),
        "bq": bq, "bk": bk, "ba": np.ascontiguousarray(ba_eff),
    }
    return shared, with_bias


def kernel(**inputs):
    shared, with_bias = _prep_inputs(inputs)
    nc = _get_nc(with_bias)
    xs = np.ascontiguousarray(np.asarray(inputs["x"], dtype=np.float32))
    in_maps = [dict(shared, x=xs[b], x8=np.ascontiguousarray(_to_f8(xs[b])))
               for b in range(B)]
    res = run_bass_kernel_spmd(nc, in_maps, core_ids=list(range(B)))
    return np.stack([res.results[b]["out"] for b in range(B)], axis=0)
